# revision 20
# baseline (speedup 1.0000x reference)
"""Cosformer (linear) attention kernel for 8 TRN2 NeuronCores.

Full (unsharded) inputs in, full output out.  Sharding: 8 cores =
4 batches x 2 head-halves.  Core c handles batch b = c//2 and heads
[hh*8, hh*8+8) where hh = c%2, i.e. embed cols [hh*512, (hh+1)*512).

Per-core math (all shapes per core):
  xT = x[:, b, :].T                        (E=1024, L=2048)  for q/k/v
  k  = relu(x_k @ Wk_s.T + bk_s)           [L, 512]  (L on partitions)
  v  =      x_v @ Wv_s.T + bv_s            [L, 512]
  qT = relu(Wq_s @ x_q.T + bq_s)           [512, L]  (head dims on partitions)
  per head h (64 dims):
    k_ = [k*sin | k*cos]                   [L, 128]
    KV_aug = k_.T @ [v | 1]                [128, 65]   (col 64 = sum_l k_)
    q2 = [qT*sin ; qT*cos] (dup via B-mm)  [128, L]
    o_aug = q2.T @ KV_aug                  [L, 65]
    o = o_aug[:, :64] / max(o_aug[:, 64], EPS)
"""

import math
from contextlib import ExitStack

import numpy as np
import ml_dtypes

BF = ml_dtypes.bfloat16

import concourse.bass as bass
import concourse.bacc as bacc_mod
import concourse.mybir as mybir
from concourse.tile import TileContext
from concourse.bass_utils import run_bass_kernel_spmd

L = 2048            # sequence length
NB = 4              # batch
E = 1024            # embed dim
D = 64              # head dim
HC = 8              # heads per core
OC = HC * D         # 512 embed cols per core
P = 128
KC = E // P         # 8 contraction chunks over E
LC = L // P         # 16 L chunks of 128
NLC = L // 512      # 4 L chunks of 512
OCC = OC // P       # 4 q-proj output chunks
EPS = 1e-4

F32 = mybir.dt.float32
F32R = mybir.dt.float32r
BF16 = mybir.dt.bfloat16
AF = mybir.ActivationFunctionType


def build_nc(with_bias=True):
    nc = bacc_mod.Bacc()

    xq = nc.declare_dram_parameter("xq", [E, L], BF16, isOutput=False)
    xk = nc.declare_dram_parameter("xk", [E, L], BF16, isOutput=False)
    xv = nc.declare_dram_parameter("xv", [E, L], BF16, isOutput=False)
    wq = nc.declare_dram_parameter("wq", [E, OC], BF16, isOutput=False)
    wk = nc.declare_dram_parameter("wk", [E, OC], BF16, isOutput=False)
    wv = nc.declare_dram_parameter("wv", [E, OC], BF16, isOutput=False)
    bqc = nc.declare_dram_parameter("bqc", [P, OCC], F32, isOutput=False)
    bkr = nc.declare_dram_parameter("bkr", [1, OC], BF16, isOutput=False)
    bvr = nc.declare_dram_parameter("bvr", [1, OC], BF16, isOutput=False)
    bmat = nc.declare_dram_parameter("bmat", [P, P], BF16, isOutput=False)
    onesr = nc.declare_dram_parameter("onesr", [1, P], BF16, isOutput=False)
    scb = nc.declare_dram_parameter("scb", [P, L], BF16, isOutput=False)
    sincol = nc.declare_dram_parameter("sincol", [P, LC], F32, isOutput=False)
    coscol = nc.declare_dram_parameter("coscol", [P, LC], F32, isOutput=False)
    outd = nc.declare_dram_parameter("out", [L, OC], F32, isOutput=True)

    xq_r = xq.rearrange("(kc p) l -> p kc l", p=P)
    xk_r = xk.rearrange("(kc p) l -> p kc l", p=P)
    xv_r = xv.rearrange("(kc p) l -> p kc l", p=P)
    out_r = outd.rearrange("(lc p) o -> lc p o", p=P)

    with TileContext(nc) as tc, ExitStack() as ctx:
        const = ctx.enter_context(tc.tile_pool(name="const", bufs=1))
        persist = ctx.enter_context(tc.tile_pool(name="persist", bufs=1))

        # DMA emission order == HWDGE issue order: front-load exactly what
        # the first matmuls need so PE starts (and HAM warms) within ~2us.
        wk_r = wk.rearrange("(kc p) o -> p kc o", p=P)
        wv_r = wv.rearrange("(kc p) o -> p kc o", p=P)
        wq_r2 = wq.rearrange("(kc p) o -> p kc o", p=P)
        wk_t = const.tile([P, KC, OC], BF16)
        wv_t = const.tile([P, KC, OC], BF16)
        wq_t = const.tile([P, KC, OC], BF16)
        nc.sync.dma_start(out=wk_t[:, 0, :], in_=wk_r[:, 0, :])
        nc.sync.dma_start(out=wv_t[:, 0, :], in_=wv_r[:, 0, :])
        bq_t = const.tile([P, OCC], F32)
        bk_t = const.tile([1, OC], BF16)
        bv_t = const.tile([1, OC], BF16)
        bm_t = const.tile([P, P], BF16)
        sin_t = const.tile([P, LC], F32)
        cos_t = const.tile([P, LC], F32)
        ones_t = const.tile([1, P], BF16)

        sc_t = persist.tile([P, L], BF16)
        qt_sb = persist.tile([P, OCC, L], BF16)      # relu(q).T  [o-dim, oc, l]
        kv_sb = persist.tile([P, HC, D + 2], BF16)   # per-head KV_aug
        q2all = persist.tile([P, HC, L], BF16)       # [qT*sin ; qT*cos] per head

        # ---------------- phase 1: projections + KV accumulation ----------
        with ExitStack() as p1:
            xkp = p1.enter_context(tc.tile_pool(name="xkp", bufs=3))
            xvp = p1.enter_context(tc.tile_pool(name="xvp", bufs=3))
            xqp = p1.enter_context(tc.tile_pool(name="xqp", bufs=1))
            kscp = p1.enter_context(tc.tile_pool(name="kscp", bufs=3))
            vap = p1.enter_context(tc.tile_pool(name="vap", bufs=3))
            pkp = p1.enter_context(tc.tile_pool(name="pkp", bufs=2, space="PSUM"))
            pvp = p1.enter_context(tc.tile_pool(name="pvp", bufs=2, space="PSUM"))
            pqp = p1.enter_context(tc.tile_pool(name="pqp", bufs=2, space="PSUM"))
            kvp = p1.enter_context(tc.tile_pool(name="kvp", bufs=1, space="PSUM"))

            kv_ps = [
                kvp.tile([P, 4, D + 2], F32, name="kv_ps0"),
                kvp.tile([P, 4, D + 2], F32, name="kv_ps1"),
            ]

            # HAM warm-up: keep PE busy during the initial DMA ramp so the
            # clock gate opens before the first real matmuls.  Results are
            # discarded (kv_ps0 is reset by the real chain's start=True).
            warm_t = kscp.tile([P, 2 * P], BF16, tag="warm", name="warm_t")
            nc.vector.memset(warm_t[:, :], 0.0)
            for w in range(16):
                nc.tensor.matmul(kv_ps[0][:, 0:2, :], warm_t[:, 0:P],
                                 warm_t[:, 0:2 * (D + 2)],
                                 start=True, stop=True)

            for lc in range(LC):
                if lc % 4 == 0:
                    xk_t4 = xkp.tile([P, KC, 4 * P], BF16, tag="xk", name="xk_t4")
                    xv_t4 = xvp.tile([P, KC, 4 * P], BF16, tag="xv", name="xv_t4")
                    if lc == 0:
                        nc.scalar.dma_start(out=xk_t4[:, :, 0:2 * P],
                                            in_=xk_r[:, :, 0:2 * P])
                        nc.scalar.dma_start(out=xv_t4[:, :, 0:2 * P],
                                            in_=xv_r[:, :, 0:2 * P])
                        nc.scalar.dma_start(out=xk_t4[:, :, 2 * P:4 * P],
                                            in_=xk_r[:, :, 2 * P:4 * P])
                        nc.scalar.dma_start(out=xv_t4[:, :, 2 * P:4 * P],
                                            in_=xv_r[:, :, 2 * P:4 * P])
                    else:
                        nc.scalar.dma_start(out=xk_t4,
                                            in_=xk_r[:, :, lc * P:(lc + 4) * P])
                        nc.scalar.dma_start(out=xv_t4,
                                            in_=xv_r[:, :, lc * P:(lc + 4) * P])
                j4 = (lc % 4) * P
                xk_t = xk_t4[:, :, j4:j4 + P]
                xv_t = xv_t4[:, :, j4:j4 + P]
                if lc == 0:
                    for kc in range(1, KC):
                        nc.sync.dma_start(out=wk_t[:, kc, :], in_=wk_r[:, kc, :])
                        nc.sync.dma_start(out=wv_t[:, kc, :], in_=wv_r[:, kc, :])
                    if with_bias:
                        nc.sync.dma_start(out=bk_t, in_=bkr[:, :])
                        nc.sync.dma_start(out=bv_t, in_=bvr[:, :])
                        nc.sync.dma_start(out=ones_t, in_=onesr[:, :])
                    nc.sync.dma_start(out=sin_t, in_=sincol[:, :])
                    nc.sync.dma_start(out=cos_t, in_=coscol[:, :])

                pk_t = pkp.tile([P, OC], F32, tag="pk", name="pk_t")
                for kc in range(KC):
                    nc.tensor.matmul(pk_t[:, :], (xk_t[:, kc, :]), (wk_t[:, kc, :]),
                                     start=(kc == 0),
                                     stop=(not with_bias and kc == KC - 1))
                if with_bias:
                    nc.tensor.matmul(pk_t[:, :], (ones_t[:, :]), (bk_t[:, :]),
                                     start=False, stop=True)

                pv_t = pvp.tile([P, OC], F32, tag="pv", name="pv_t")
                for kc in range(KC):
                    nc.tensor.matmul(pv_t[:, :], (xv_t[:, kc, :]), (wv_t[:, kc, :]),
                                     start=(kc == 0),
                                     stop=(not with_bias and kc == KC - 1))
                if with_bias:
                    nc.tensor.matmul(pv_t[:, :], (ones_t[:, :]), (bv_t[:, :]),
                                     start=False, stop=True)

                # k_sc[p, h, 0, :] = relu(k)*sin_l ; k_sc[p, h, 1, :] = relu(k)*cos_l
                # (sin/cos >= 0 on (0, pi/2], so relu(k*s) == relu(k)*s)
                ksc_t = kscp.tile([P, HC, 2, D], BF16, tag="ksc", name="ksc_t")
                pk_v = pk_t.rearrange("p (h d) -> p h d", d=D)
                nc.scalar.activation(ksc_t[:, :, 0, :], pk_v, AF.Relu,
                                     scale=sin_t[:, lc:lc + 1])
                nc.scalar.activation(ksc_t[:, :, 1, :], pk_v, AF.Relu,
                                     scale=cos_t[:, lc:lc + 1])

                va_t = vap.tile([P, HC, D + 2], BF16, tag="va", name="va_t")
                pv_v = pv_t.rearrange("p (h d) -> p h d", d=D)
                nc.scalar.activation(va_t[:, :, D:D + 2], pv_v[:, :, 0:2],
                                     AF.Copy, bias=1.0, scale=0.0)
                nc.vector.tensor_copy(va_t[:, :, 0:D], pv_v)

                # KV_aug accumulation: 4 heads share one PSUM bank; only the
                # very first matmul into each bank uses start=True (clears
                # has_written bank-wide), everything else start=False so the
                # per-element has_written bits do the right thing.
                for h in range(HC):
                    nc.tensor.matmul(
                        kv_ps[h // 4][:, h % 4, :],
                        (ksc_t[:, h, :, :]),
                        (va_t[:, h, :]),
                        start=(lc == 0 and h % 4 == 0),
                        stop=(lc == LC - 1 and h % 4 == 3),
                    )

            # q projection (transposed layout); oc-outer so each oc chunk of
            # qt_sb completes early and the q2 matmuls can overlap
            nc.sync.dma_start(out=bm_t, in_=bmat[:, :])
            nc.sync.dma_start(out=sc_t, in_=scb[:, :])
            nc.sync.dma_start(out=wq_t, in_=wq_r2)
            nc.sync.dma_start(out=bq_t, in_=bqc[:, :])
            xq_ts = []
            for nlc in range(NLC):
                xq_t = xqp.tile([P, KC, 512], BF16, tag=f"xq{nlc}", name="xq_t")
                nc.sync.dma_start(out=xq_t, in_=xq_r[:, :, nlc * 512:(nlc + 1) * 512])
                xq_ts.append(xq_t)
            for oc in range(OCC):
                for nlc in range(NLC):
                    pq_t = pqp.tile([P, 512], F32, tag="pq", name="pq_t")
                    for kc in range(KC):
                        nc.tensor.matmul(
                            pq_t[:, :],
                            (wq_t[:, kc, oc * P:(oc + 1) * P]),
                            (xq_ts[nlc][:, kc, :]),
                            start=(kc == 0), stop=(kc == KC - 1))
                    nc.scalar.activation(
                        qt_sb[:, oc, nlc * 512:(nlc + 1) * 512], pq_t[:, :],
                        AF.Relu, bias=bq_t[:, oc:oc + 1])

            # build q2 = [qT*sin ; qT*cos] for all 8 heads (overlaps q-proj;
            # psum slots shared with the q-projection pool)
            for h in range(HC):
                occ, half = divmod(h, 2)
                for nlc in range(NLC):
                    pq2_t = pqp.tile([P, 512], F32, tag="pq", name="pq2_t")
                    nc.tensor.matmul(
                        pq2_t[:, :],
                        (bm_t[half * D:(half + 1) * D, :]),
                        (qt_sb[half * D:(half + 1) * D, occ,
                               nlc * 512:(nlc + 1) * 512]),
                        start=True, stop=True)
                    nc.vector.tensor_mul(q2all[:, h, nlc * 512:(nlc + 1) * 512],
                                         pq2_t[:, :],
                                         sc_t[:, nlc * 512:(nlc + 1) * 512])

            # evict KV accumulators to SBUF
            nc.vector.tensor_copy(kv_sb[:, 0:4, :], kv_ps[0][:, :, :])
            nc.vector.tensor_copy(kv_sb[:, 4:8, :], kv_ps[1][:, :, :])

        # ---------------- phase 3: attention output ------------------------
        with ExitStack() as p3:
            osbp = p3.enter_context(tc.tile_pool(name="osbp", bufs=3))
            zp = p3.enter_context(tc.tile_pool(name="zp", bufs=4))
            pop = p3.enter_context(tc.tile_pool(name="pop", bufs=4, space="PSUM"))

            # lc-major attention: all 8 heads per L-chunk, stream output
            for lc in range(LC):
                o_t = osbp.tile([P, OC], F32, tag="osb", name="o_t")
                for hg in range(2):
                    po_t = pop.tile([P, 4, D + 2], F32, tag="po", name="po_t")
                    for j in range(4):
                        h = hg * 4 + j
                        nc.tensor.matmul(po_t[:, j, :],
                                         (q2all[:, h, lc * P:(lc + 1) * P]),
                                         (kv_sb[:, h, :]),
                                         start=True, stop=True)
                    z_t = zp.tile([P, 4], F32, tag="z", name="z_t")
                    zr_t = zp.tile([P, 4], F32, tag="zr", name="zr_t")
                    nc.vector.tensor_scalar_max(z_t[:, :], po_t[:, :, D], EPS)
                    nc.vector.reciprocal(zr_t[:, :], z_t[:, :])
                    zb = zr_t[:, :].unsqueeze(2).broadcast_to((P, 4, D))
                    ov = o_t.rearrange("p (h d) -> p h d", d=D)
                    nc.vector.tensor_tensor(ov[:, hg * 4:(hg + 1) * 4, :],
                                            po_t[:, :, 0:D], zb,
                                            mybir.AluOpType.mult)
                nc.sync.dma_start(out=out_r[lc], in_=o_t[:, :])

    nc.compile()
    return nc


_NC = {}


def _get_nc(with_bias=True):
    if with_bias not in _NC:
        _NC[with_bias] = build_nc(with_bias)
    return _NC[with_bias]


def _host_constants():
    idx = (math.pi / 2.0) * (np.arange(L, dtype=np.float64) + 1.0) / float(L)
    sinv = np.sin(idx).astype(np.float32)
    cosv = np.cos(idx).astype(np.float32)
    scb = np.empty((P, L), np.float32)
    scb[0:D, :] = sinv[None, :]
    scb[D:P, :] = cosv[None, :]
    eye2 = np.concatenate([np.eye(D, dtype=np.float32)] * 2, axis=1)  # [64, 128]
    bmat = np.concatenate([eye2, eye2], axis=0)                       # [128, 128]
    return {
        "scb": scb.astype(BF),
        "sincol": np.ascontiguousarray(sinv.reshape(LC, P).T),
        "coscol": np.ascontiguousarray(cosv.reshape(LC, P).T),
        "bmat": np.ascontiguousarray(bmat).astype(BF),
        "onesr": np.ones((1, P), BF),
    }


def kernel(query, key, value, Wq, bq, Wk, bk, Wv, bv):
    query = np.asarray(query, np.float32)
    key = np.asarray(key, np.float32)
    value = np.asarray(value, np.float32)
    Wq = np.asarray(Wq, np.float32)
    Wk = np.asarray(Wk, np.float32)
    Wv = np.asarray(Wv, np.float32)
    bq = np.asarray(bq, np.float32)
    bk = np.asarray(bk, np.float32)
    bv = np.asarray(bv, np.float32)

    consts = _host_constants()
    in_maps = []
    for c in range(8):
        b, hh = divmod(c, 2)
        sl = slice(hh * OC, (hh + 1) * OC)
        in_maps.append({
            "xq": np.ascontiguousarray(query[:, b, :].T).astype(BF),
            "xk": np.ascontiguousarray(key[:, b, :].T).astype(BF),
            "xv": np.ascontiguousarray(value[:, b, :].T).astype(BF),
            "wq": np.ascontiguousarray(Wq[sl, :].T).astype(BF),
            "wk": np.ascontiguousarray(Wk[sl, :].T).astype(BF),
            "wv": np.ascontiguousarray(Wv[sl, :].T).astype(BF),
            "bqc": np.ascontiguousarray(bq[sl].reshape(OCC, P).T),
            "bkr": np.ascontiguousarray(bk[sl].reshape(1, OC)).astype(BF),
            "bvr": np.ascontiguousarray(bv[sl].reshape(1, OC)).astype(BF),
            **consts,
        })

    with_bias = bool(np.any(bk) or np.any(bv))
    res = run_bass_kernel_spmd(_get_nc(with_bias), in_maps,
                               core_ids=list(range(8))).results

    out = np.empty((L, NB, E), np.float32)
    for c in range(8):
        b, hh = divmod(c, 2)
        out[:, b, hh * OC:(hh + 1) * OC] = res[c]["out"]
    return out


if __name__ == "__main__":
    nc = build_nc()
    print("build OK")
